# revision 10
# baseline (speedup 1.0000x reference)
"""Causal multi-head attention (B=2, S=2048, D=1024, H=16, d_k=64) on 8
Trainium2 NeuronCores.

Sharding: tensor-parallel over heads x data-parallel over batch.
Core (b*4 + c) computes batch b, heads 4c..4c+3 (a 256-wide d-slice):
  Q^T/K^T = W_slice @ x^T              (d-major, f32r)
  V       = x @ Wv_slice^T             (seq-major, f32r, + ones column)
  S^T     = K^T x Q^T matmul           ([k,q] tiles, causal tiles only)
  P^T     = exp(S^T/8) (+ diag mask)   (ACT, f32r out)
  A^T,l   = [V|1] x P^T matmul         (accumulated in PSUM; row 64 = l)
  A^T    /= l                          (ACT copy + gpsimd bcast + DVE)
  out_c   = A^T.T @ Wo_slice^T         (partial over the d-slice)
Host sums the 4 partial outputs per batch (the W_o row-parallel reduce).

All matmul inputs are float32r (TF32): full PE rate at N>=256 with
~1.5e-4 matmul relative error (measured on HW).
"""

import numpy as np

import concourse.bass as bass
import concourse.mybir as mybir
from concourse import bacc
from concourse.tile import TileContext
from concourse.bass_utils import run_bass_kernel_spmd

F32 = mybir.dt.float32
F32R = mybir.dt.float32r
Exp = mybir.ActivationFunctionType.Exp
Copy = mybir.ActivationFunctionType.Copy
Ln = mybir.ActivationFunctionType.Ln

B = 2
S = 2048
D = 1024
HEADS_PER_CORE = 4
DC = 64 * HEADS_PER_CORE  # 256: d-slice width per core
N_CORES = 8
NEG = -1.0e9


def _round_tf32(x: np.ndarray) -> np.ndarray:
    i = np.ascontiguousarray(x).view(np.uint32)
    return ((i + 0x1000) & 0xFFFFE000).view(np.float32)


def _build_mask() -> np.ndarray:
    """Triangular additive mask for a diagonal 128x128 tile:
    tri[k', r] = 0 if k' <= r else NEG."""
    kp = np.arange(128)[:, None]
    r = np.arange(128)[None, :]
    return np.where(kp <= r, 0.0, NEG).astype(np.float32)


_COMBINED_SET = "natural_log_exp_and_others"


def _patch_act_tables():
    """Exp and Ln live in different default table sets; interleaving them
    makes the table-load pass thrash (~3.5us per switch). Restrict both to
    the one set that contains them together."""
    orig = bacc.get_activation_tables
    if getattr(bacc.get_activation_tables, "_mha_patched", False):
        return

    def patched(arch):
        tables = orig(arch)
        for name, fns in tables.items():
            if name != _COMBINED_SET:
                fns.discard(mybir.ActivationFunctionType.Exp)
                fns.discard(mybir.ActivationFunctionType.Ln)
        return tables

    patched._mha_patched = True
    bacc.get_activation_tables = patched


def _build_nc():
    _patch_act_tables()
    nc = bacc.Bacc("TRN2", target_bir_lowering=False)
    xT = nc.declare_dram_parameter("xT", [D, S], F32R, isOutput=False)
    wqT = nc.declare_dram_parameter("wqT", [D, DC], F32R, isOutput=False)
    wkT = nc.declare_dram_parameter("wkT", [D, DC], F32R, isOutput=False)
    wvT = nc.declare_dram_parameter("wvT", [D, DC], F32R, isOutput=False)
    woT = nc.declare_dram_parameter("woT", [DC, D], F32R, isOutput=False)
    maskd = nc.declare_dram_parameter("mask", [128, 128], F32, isOutput=False)
    outd = nc.declare_dram_parameter("out", [S, D], F32, isOutput=True)

    with TileContext(nc) as tc:
        with (
            tc.tile_pool(name="persist", bufs=1) as pp,
            tc.tile_pool(name="vpool", bufs=1) as vp,
        ):
            # persistent tiles
            QT = pp.tile([64, 4 * S], F32R, name="QT")  # 4 heads concat on free
            KT = pp.tile([64, 4 * S], F32R, name="KT")
            woT_sb = [pp.tile([128, D], F32R, name=f"wo{t}") for t in range(2)]
            mask_sb = pp.tile([128, 128], F32, name="mask")
            ones_sb = pp.tile([128, 1], F32, name="ones")
            V_sb = [vp.tile([128, 65 * 4], F32R, name=f"V{kt}") for kt in range(16)]

            # ---------------- Phase 1: projections ----------------
            with (
                tc.tile_pool(name="xw", bufs=1) as xw,
                tc.tile_pool(name="psQ", bufs=8, space="PSUM") as psQp,
            ):
                xT_sb = []
                wq_sb = []
                wk_sb = []
                wv_sb = []
                # DMA issue order matches Dt-outer consumption
                for dt in range(8):
                    r = slice(128 * dt, 128 * dt + 128)
                    xt = xw.tile([128, S], F32R, name=f"x{dt}")
                    nc.sync.dma_start(xt[:], xT[r, :])
                    xT_sb.append(xt)
                    for nm, dram, lst in (
                        ("q", wqT, wq_sb),
                        ("k", wkT, wk_sb),
                        ("v", wvT, wv_sb),
                    ):
                        wt = xw.tile([128, DC], F32R, name=f"w{nm}{dt}")
                        nc.sync.dma_start(wt[:], dram[r, :])
                        lst.append(wt)
                for t in range(2):
                    nc.sync.dma_start(woT_sb[t][:], woT[128 * t : 128 * t + 128, :])
                nc.sync.dma_start(mask_sb[:], maskd[:])
                nc.vector.memset(ones_sb[:], 1.0)

                # Q^T: Dt-outer over 8 live psum banks so PE starts on the
                # first x tile instead of waiting for the whole x DMA.
                psq = [
                    psQp.tile([128, 512], F32, name=f"psq{i}", tag="psq")
                    for i in range(8)
                ]
                for dt in range(8):
                    for t in range(2):
                        for j in range(4):
                            nc.tensor.matmul(
                                psq[4 * t + j][:],
                                wq_sb[dt][:, 128 * t : 128 * t + 128],
                                xT_sb[dt][:, 512 * j : 512 * j + 512],
                                start=(dt == 0),
                                stop=(dt == 7),
                            )
                for t in range(2):
                    for j in range(4):
                        for hh in range(2):
                            h = 2 * t + hh
                            nc.vector.tensor_copy(
                                QT[:, S * h + 512 * j : S * h + 512 * j + 512],
                                psq[4 * t + j][64 * hh : 64 * hh + 64, :],
                            )

                # K^T: same, second rotation of the 8 banks
                psk = [
                    psQp.tile([128, 512], F32, name=f"psk{i}", tag="psq")
                    for i in range(8)
                ]
                for dt in range(8):
                    for t in range(2):
                        for j in range(4):
                            nc.tensor.matmul(
                                psk[4 * t + j][:],
                                wk_sb[dt][:, 128 * t : 128 * t + 128],
                                xT_sb[dt][:, 512 * j : 512 * j + 512],
                                start=(dt == 0),
                                stop=(dt == 7),
                            )
                for t in range(2):
                    for j in range(4):
                        for hh in range(2):
                            h = 2 * t + hh
                            nc.vector.tensor_copy(
                                KT[:, S * h + 512 * j : S * h + 512 * j + 512],
                                psk[4 * t + j][64 * hh : 64 * hh + 64, :],
                            )

                # V (seq-major) + ones columns; all x resident by now
                for kt in range(16):
                    ps = psQp.tile([128, DC], F32, name=f"psv{kt}", tag="psq")
                    for dt in range(8):
                        nc.tensor.matmul(
                            ps[:],
                            xT_sb[dt][:, 128 * kt : 128 * kt + 128],
                            wv_sb[dt][:],
                            start=(dt == 0),
                            stop=(dt == 7),
                        )
                    for h in range(4):
                        nc.vector.tensor_copy(
                            V_sb[kt][:, 65 * h : 65 * h + 64],
                            ps[:, 64 * h : 64 * h + 64],
                        )
                        nc.vector.tensor_copy(
                            V_sb[kt][:, 65 * h + 64 : 65 * h + 65], ones_sb[:]
                        )

            # ---------------- Phase 2+3: attention + out-proj ----------------
            # PSUM: psS pool 2 slots x [128,1024] = 4 banks;
            #       shared psA/psO pool 4 slots x 1 bank = 4 banks.
            with (
                tc.tile_pool(name="atp", bufs=1) as atp,
                tc.tile_pool(name="psS", bufs=2, space="PSUM") as psSp,
                tc.tile_pool(name="psA", bufs=4, space="PSUM") as psAp,
                tc.tile_pool(name="pt", bufs=3) as ptp,
                tc.tile_pool(name="norm", bufs=3) as nrm,
                tc.tile_pool(name="osb", bufs=3) as op_,
            ):
                AT = [atp.tile([128, S], F32R, name=f"AT{t}") for t in range(2)]
                pending = []  # deferred normalize/out-proj emitters

                def flush_pending():
                    for fn in pending:
                        fn()
                    pending.clear()

                def make_norm(psA_, j_, heads_):
                    def emit():
                        for h in heads_:
                            ls = nrm.tile([1, 512], F32, name=f"ls{h}{j_}", tag="ls")
                            nc.vector.tensor_copy(ls[:], psA_[h][64:65, :])
                            lr = nrm.tile([1, 512], F32, name=f"lr{h}{j_}", tag="lr")
                            nc.vector.reciprocal_approx_fast(lr[:], ls[:])
                            rb = nrm.tile([128, 512], F32, name=f"rb{h}{j_}", tag="rb")
                            nc.gpsimd.partition_broadcast(rb[:], lr[:])
                            t, hh = divmod(h, 2)
                            po = 64 * hh
                            nc.vector.tensor_mul(
                                AT[t][po : po + 64, 512 * j_ : 512 * j_ + 512],
                                psA_[h][0:64, :],
                                rb[po : po + 64, :],
                            )

                    return emit

                def make_outproj(j_):
                    def emit():
                        for qt in range(4 * j_, 4 * j_ + 4):
                            for mc in range(2):
                                psO = psAp.tile(
                                    [128, 512], F32, name=f"pso{qt}{mc}", tag="psa"
                                )
                                for t in range(2):
                                    nc.tensor.matmul(
                                        psO[:],
                                        AT[t][:, 128 * qt : 128 * qt + 128],
                                        woT_sb[t][:, 512 * mc : 512 * mc + 512],
                                        start=(t == 0),
                                        stop=(t == 1),
                                    )
                                ot = op_.tile(
                                    [128, 512], F32, name=f"ot{qt}{mc}", tag="ot"
                                )
                                nc.vector.tensor_copy(ot[:], psO[:])
                                nc.sync.dma_start(
                                    outd[
                                        128 * qt : 128 * qt + 128,
                                        512 * mc : 512 * mc + 512,
                                    ],
                                    ot[:],
                                )

                    return emit

                for j in range(4):  # q-chunk of 512
                    npairs = 2 * j + 2
                    last_kt = 4 * j + 3
                    for hp in range(2):  # head pair
                        heads = (2 * hp, 2 * hp + 1)
                        psA = {}
                        pts = {}

                        def emit_pv(p, psA=psA, pts=pts, heads=heads, j=j,
                                    last_kt=last_kt):
                            diag = p >= 2 * j
                            for h in heads:
                                for t2 in range(2):
                                    kt = 2 * p + t2
                                    # diagonal tiles: q-cols < 128*tg are fully
                                    # masked (zero P) -> skip in the matmul
                                    lo = (
                                        128 * (2 * (p - 2 * j) + t2) if diag else 0
                                    )
                                    nc.tensor.matmul(
                                        psA[h][:, lo:],
                                        V_sb[kt][:, 65 * h : 65 * h + 65],
                                        pts[(h, p)][:, 512 * t2 + lo : 512 * t2 + 512],
                                        start=(kt == 0),
                                        stop=(kt == last_kt),
                                    )

                        for p in range(npairs):
                            for h in heads:
                                psS = psSp.tile(
                                    [128, 1024], F32, name=f"pss{h}_{j}_{p}",
                                    tag="pss",
                                )
                                for t2 in range(2):
                                    kt = 2 * p + t2
                                    nc.tensor.matmul(
                                        psS[:, 512 * t2 : 512 * t2 + 512],
                                        KT[:, S * h + 128 * kt : S * h + 128 * kt + 128],
                                        QT[:, S * h + 512 * j : S * h + 512 * j + 512],
                                        start=True,
                                        stop=True,
                                    )
                                diag = p >= 2 * j
                                if diag:  # triangular mask on the 2 diag tiles
                                    for t2 in range(2):
                                        tg = 2 * (p - 2 * j) + t2
                                        lo = 512 * t2 + 128 * tg
                                        nc.vector.tensor_add(
                                            psS[:, lo : lo + 128],
                                            psS[:, lo : lo + 128],
                                            mask_sb[:],
                                        )
                                pt = ptp.tile(
                                    [128, 1024], F32R, name=f"pt{h}_{j}_{p}",
                                    tag=f"pt{h % 2}",
                                )
                                nc.scalar.activation(pt[:], psS[:], Exp, scale=0.125)
                                pts[(h, p)] = pt
                            if p == 0:
                                flush_pending()  # prev chunk norm / out-proj
                                for h in heads:  # lazy alloc: after flush so
                                    # slot rotation follows program order
                                    psA[h] = psAp.tile(
                                        [65, 512], F32, name=f"psa{h}_{j}",
                                        tag="psa",
                                    )
                            else:  # 1-stage skew: PV of previous pair
                                emit_pv(p - 1)
                        emit_pv(npairs - 1)
                        pending.append(make_norm(psA, j, heads))

                    pending.append(make_outproj(j))
                flush_pending()

    nc.finalize()
    return nc


_NC = None


def _get_nc():
    global _NC
    if _NC is None:
        _NC = _build_nc()
    return _NC


def kernel(x, W_q, W_k, W_v, W_o):
    nc = _get_nc()
    mask = _build_mask()
    in_maps = []
    xTs = [_round_tf32(x[b].T) for b in range(B)]
    for core in range(N_CORES):
        b, c = divmod(core, 4)
        sl = slice(DC * c, DC * c + DC)
        in_maps.append(
            {
                "xT": xTs[b],
                "wqT": _round_tf32(W_q[sl, :].T),
                "wkT": _round_tf32(W_k[sl, :].T),
                "wvT": _round_tf32(W_v[sl, :].T),
                "woT": _round_tf32(W_o[:, sl].T),
                "mask": mask,
            }
        )
    res = run_bass_kernel_spmd(nc, in_maps, list(range(N_CORES)))
    outs = [res.results[i]["out"] for i in range(N_CORES)]
    full = np.stack(
        [outs[0] + outs[1] + outs[2] + outs[3], outs[4] + outs[5] + outs[6] + outs[7]]
    )
    return full.astype(np.float32)


# revision 15
# speedup vs baseline: 1.0338x; 1.0338x over previous
"""Causal multi-head attention (B=2, S=2048, D=1024, H=16, d_k=64) on 8
Trainium2 NeuronCores.

Sharding: tensor-parallel over heads x data-parallel over batch.
Core (b*4 + c) computes batch b, heads 4c..4c+3 (a 256-wide d-slice):
  Q^T/K^T = W_slice @ x^T              (d-major, f32r)
  V       = x @ Wv_slice^T             (seq-major, f32r, + ones column)
  S^T     = K^T x Q^T matmul           ([k,q] tiles, causal tiles only)
  P^T     = exp(S^T/8) (+ diag mask)   (ACT, f32r out)
  A^T,l   = [V|1] x P^T matmul         (accumulated in PSUM; row 64 = l)
  A^T    /= l                          (ACT copy + gpsimd bcast + DVE)
  out_c   = A^T.T @ Wo_slice^T         (partial over the d-slice)
Host sums the 4 partial outputs per batch (the W_o row-parallel reduce).

All matmul inputs are float32r (TF32): full PE rate at N>=256 with
~1.5e-4 matmul relative error (measured on HW).
"""

import numpy as np

import concourse.bass as bass
import concourse.mybir as mybir
from concourse import bacc
from concourse.tile import TileContext
from concourse.bass_utils import run_bass_kernel_spmd

F32 = mybir.dt.float32
F32R = mybir.dt.float32r
Exp = mybir.ActivationFunctionType.Exp
Copy = mybir.ActivationFunctionType.Copy
Ln = mybir.ActivationFunctionType.Ln

B = 2
S = 2048
D = 1024
HEADS_PER_CORE = 4
DC = 64 * HEADS_PER_CORE  # 256: d-slice width per core
N_CORES = 8
NEG = -1.0e9


def _round_tf32(x: np.ndarray) -> np.ndarray:
    i = np.ascontiguousarray(x).view(np.uint32)
    return ((i + 0x1000) & 0xFFFFE000).view(np.float32)


def _build_mask() -> np.ndarray:
    """Triangular additive mask for a diagonal 128x128 tile:
    tri[k', r] = 0 if k' <= r else NEG."""
    kp = np.arange(128)[:, None]
    r = np.arange(128)[None, :]
    return np.where(kp <= r, 0.0, NEG).astype(np.float32)


_COMBINED_SET = "natural_log_exp_and_others"


def _patch_act_tables():
    """Exp and Ln live in different default table sets; interleaving them
    makes the table-load pass thrash (~3.5us per switch). Restrict both to
    the one set that contains them together."""
    orig = bacc.get_activation_tables
    if getattr(bacc.get_activation_tables, "_mha_patched", False):
        return

    def patched(arch):
        tables = orig(arch)
        for name, fns in tables.items():
            if name != _COMBINED_SET:
                fns.discard(mybir.ActivationFunctionType.Exp)
                fns.discard(mybir.ActivationFunctionType.Ln)
        return tables

    patched._mha_patched = True
    bacc.get_activation_tables = patched


def _build_nc():
    _patch_act_tables()
    nc = bacc.Bacc("TRN2", target_bir_lowering=False)
    xT = nc.declare_dram_parameter("xT", [D, S], F32R, isOutput=False)
    wqT = nc.declare_dram_parameter("wqT", [D, DC], F32R, isOutput=False)
    wkT = nc.declare_dram_parameter("wkT", [D, DC], F32R, isOutput=False)
    wvT = nc.declare_dram_parameter("wvT", [D, DC], F32R, isOutput=False)
    woT = nc.declare_dram_parameter("woT", [DC, D], F32R, isOutput=False)
    maskd = nc.declare_dram_parameter("mask", [128, 128], F32, isOutput=False)
    outd = nc.declare_dram_parameter("out", [S, D], F32, isOutput=True)

    with TileContext(nc) as tc:
        with (
            tc.tile_pool(name="persist", bufs=1) as pp,
            tc.tile_pool(name="vpool", bufs=1) as vp,
        ):
            # persistent tiles
            QT = pp.tile([64, 4 * S], F32R, name="QT")  # 4 heads concat on free
            KT = pp.tile([64, 4 * S], F32R, name="KT")
            woT_sb = [pp.tile([128, D], F32R, name=f"wo{t}") for t in range(2)]
            mask_sb = pp.tile([128, 128], F32, name="mask")
            ones_sb = pp.tile([128, 1], F32, name="ones")
            V_sb = [vp.tile([128, 65 * 4], F32R, name=f"V{kt}") for kt in range(16)]

            # ---------------- Phase 1: projections ----------------
            with (
                tc.tile_pool(name="xw", bufs=1) as xw,
                tc.tile_pool(name="psQ", bufs=8, space="PSUM") as psQp,
            ):
                xT_sb = []
                wq_sb = []
                wk_sb = []
                wv_sb = []
                # DMA issue order matches Dt-outer consumption
                for dt in range(8):
                    r = slice(128 * dt, 128 * dt + 128)
                    xt = xw.tile([128, S], F32R, name=f"x{dt}")
                    nc.sync.dma_start(xt[:], xT[r, :])
                    xT_sb.append(xt)
                    for nm, dram, lst in (
                        ("q", wqT, wq_sb),
                        ("k", wkT, wk_sb),
                        ("v", wvT, wv_sb),
                    ):
                        wt = xw.tile([128, DC], F32R, name=f"w{nm}{dt}")
                        nc.sync.dma_start(wt[:], dram[r, :])
                        lst.append(wt)
                for t in range(2):
                    nc.sync.dma_start(woT_sb[t][:], woT[128 * t : 128 * t + 128, :])
                nc.sync.dma_start(mask_sb[:], maskd[:])
                nc.vector.memset(ones_sb[:], 1.0)

                # Q^T: Dt-outer over 8 live psum banks so PE starts on the
                # first x tile instead of waiting for the whole x DMA.
                psq = [
                    psQp.tile([128, 512], F32, name=f"psq{i}", tag="psq")
                    for i in range(8)
                ]
                for dt in range(8):
                    for t in range(2):
                        for j in range(4):
                            nc.tensor.matmul(
                                psq[4 * t + j][:],
                                wq_sb[dt][:, 128 * t : 128 * t + 128],
                                xT_sb[dt][:, 512 * j : 512 * j + 512],
                                start=(dt == 0),
                                stop=(dt == 7),
                            )
                for t in range(2):
                    for j in range(4):
                        for hh in range(2):
                            h = 2 * t + hh
                            nc.vector.tensor_copy(
                                QT[:, S * h + 512 * j : S * h + 512 * j + 512],
                                psq[4 * t + j][64 * hh : 64 * hh + 64, :],
                            )

                # K^T: same, second rotation of the 8 banks
                psk = [
                    psQp.tile([128, 512], F32, name=f"psk{i}", tag="psq")
                    for i in range(8)
                ]
                for dt in range(8):
                    for t in range(2):
                        for j in range(4):
                            nc.tensor.matmul(
                                psk[4 * t + j][:],
                                wk_sb[dt][:, 128 * t : 128 * t + 128],
                                xT_sb[dt][:, 512 * j : 512 * j + 512],
                                start=(dt == 0),
                                stop=(dt == 7),
                            )
                for t in range(2):
                    for j in range(4):
                        for hh in range(2):
                            h = 2 * t + hh
                            nc.vector.tensor_copy(
                                KT[:, S * h + 512 * j : S * h + 512 * j + 512],
                                psk[4 * t + j][64 * hh : 64 * hh + 64, :],
                            )

                # V (seq-major) + ones columns; all x resident by now
                for kt in range(16):
                    ps = psQp.tile([128, DC], F32, name=f"psv{kt}", tag="psq")
                    for dt in range(8):
                        nc.tensor.matmul(
                            ps[:],
                            xT_sb[dt][:, 128 * kt : 128 * kt + 128],
                            wv_sb[dt][:],
                            start=(dt == 0),
                            stop=(dt == 7),
                        )
                    for h in range(4):
                        nc.vector.tensor_copy(
                            V_sb[kt][:, 65 * h : 65 * h + 64],
                            ps[:, 64 * h : 64 * h + 64],
                        )
                        nc.vector.tensor_copy(
                            V_sb[kt][:, 65 * h + 64 : 65 * h + 65], ones_sb[:]
                        )

            # ---------------- Phase 2+3: attention + out-proj ----------------
            # PSUM: psS pool 2 slots x [128,1024] = 4 banks;
            #       shared psA/psO pool 4 slots x 1 bank = 4 banks.
            with (
                tc.tile_pool(name="atp", bufs=1) as atp,
                tc.tile_pool(name="psS", bufs=2, space="PSUM") as psSp,
                tc.tile_pool(name="psA", bufs=4, space="PSUM") as psAp,
                tc.tile_pool(name="pt", bufs=3) as ptp,
                tc.tile_pool(name="norm", bufs=3) as nrm,
                tc.tile_pool(name="osb", bufs=3) as op_,
            ):
                AT = [atp.tile([128, S], F32R, name=f"AT{t}") for t in range(2)]
                pending = []  # deferred normalize/out-proj emitters

                def flush_pending():
                    for fn in pending:
                        fn()
                    pending.clear()

                def make_norm(psA_, j_, heads_):
                    def emit():
                        for h in heads_:
                            ls = nrm.tile([1, 512], F32, name=f"ls{h}{j_}", tag="ls")
                            nc.vector.tensor_copy(ls[:], psA_[h][64:65, :])
                            lr = nrm.tile([1, 512], F32, name=f"lr{h}{j_}", tag="lr")
                            nc.vector.reciprocal_approx_fast(lr[:], ls[:])
                            rb = nrm.tile([128, 512], F32, name=f"rb{h}{j_}", tag="rb")
                            nc.gpsimd.partition_broadcast(rb[:], lr[:])
                            t, hh = divmod(h, 2)
                            po = 64 * hh
                            nc.vector.tensor_mul(
                                AT[t][po : po + 64, 512 * j_ : 512 * j_ + 512],
                                psA_[h][0:64, :],
                                rb[po : po + 64, :],
                            )

                    return emit

                def make_outproj(j_):
                    def emit():
                        for qt in range(4 * j_, 4 * j_ + 4):
                            for mc in range(2):
                                psO = psAp.tile(
                                    [128, 512], F32, name=f"pso{qt}{mc}", tag="psa"
                                )
                                for t in range(2):
                                    nc.tensor.matmul(
                                        psO[:],
                                        AT[t][:, 128 * qt : 128 * qt + 128],
                                        woT_sb[t][:, 512 * mc : 512 * mc + 512],
                                        start=(t == 0),
                                        stop=(t == 1),
                                    )
                                ot = op_.tile(
                                    [128, 512], F32, name=f"ot{qt}{mc}", tag="ot"
                                )
                                nc.vector.tensor_copy(ot[:], psO[:])
                                nc.sync.dma_start(
                                    outd[
                                        128 * qt : 128 * qt + 128,
                                        512 * mc : 512 * mc + 512,
                                    ],
                                    ot[:],
                                )

                    return emit

                for j in range(4):  # q-chunk of 512
                    npairs = 2 * j + 2
                    last_kt = 4 * j + 3
                    for hp in range(2):  # head pair
                        heads = (2 * hp, 2 * hp + 1)
                        psA = {}
                        pts = {}

                        def emit_pv(p, psA=psA, pts=pts, heads=heads, j=j,
                                    last_kt=last_kt):
                            diag = p >= 2 * j
                            for h in heads:
                                for t2 in range(2):
                                    kt = 2 * p + t2
                                    # diagonal tiles: q-cols < 128*tg are fully
                                    # masked (zero P) -> skip in the matmul
                                    lo = (
                                        128 * (2 * (p - 2 * j) + t2) if diag else 0
                                    )
                                    nc.tensor.matmul(
                                        psA[h][:, lo:],
                                        V_sb[kt][:, 65 * h : 65 * h + 65],
                                        pts[(h, p)][:, 512 * t2 + lo : 512 * t2 + 512],
                                        start=(kt == 0),
                                        stop=(kt == last_kt),
                                    )

                        for p in range(npairs):
                            psS = {}
                            for h in heads:
                                psS[h] = psSp.tile(
                                    [128, 1024], F32, name=f"pss{h}_{j}_{p}",
                                    tag="pss",
                                )
                            for t2 in range(2):
                                kt = 2 * p + t2
                                for h in heads:
                                    nc.tensor.matmul(
                                        psS[h][:, 512 * t2 : 512 * t2 + 512],
                                        KT[:, S * h + 128 * kt : S * h + 128 * kt + 128],
                                        QT[:, S * h + 512 * j : S * h + 512 * j + 512],
                                        start=True,
                                        stop=True,
                                    )
                            diag = p >= 2 * j
                            for h in heads:
                                if diag:  # triangular mask on the 2 diag tiles
                                    for t2 in range(2):
                                        tg = 2 * (p - 2 * j) + t2
                                        lo = 512 * t2 + 128 * tg
                                        nc.vector.tensor_add(
                                            psS[h][:, lo : lo + 128],
                                            psS[h][:, lo : lo + 128],
                                            mask_sb[:],
                                        )
                                pt = ptp.tile(
                                    [128, 1024], F32R, name=f"pt{h}_{j}_{p}",
                                    tag=f"pt{h % 2}",
                                )
                                nc.scalar.activation(
                                    pt[:], psS[h][:], Exp, scale=0.125
                                )
                                pts[(h, p)] = pt
                            if p == 0:
                                flush_pending()  # prev chunk norm / out-proj
                                for h in heads:  # lazy alloc: after flush so
                                    # slot rotation follows program order
                                    psA[h] = psAp.tile(
                                        [65, 512], F32, name=f"psa{h}_{j}",
                                        tag="psa",
                                    )
                            else:  # 1-stage skew: PV of previous pair
                                emit_pv(p - 1)
                        emit_pv(npairs - 1)
                        pending.append(make_norm(psA, j, heads))

                    pending.append(make_outproj(j))
                flush_pending()

    nc.finalize()
    return nc


_NC = None


def _get_nc():
    global _NC
    if _NC is None:
        _NC = _build_nc()
    return _NC


def kernel(x, W_q, W_k, W_v, W_o):
    nc = _get_nc()
    mask = _build_mask()
    in_maps = []
    xTs = [_round_tf32(x[b].T) for b in range(B)]
    for core in range(N_CORES):
        b, c = divmod(core, 4)
        sl = slice(DC * c, DC * c + DC)
        in_maps.append(
            {
                "xT": xTs[b],
                "wqT": _round_tf32(W_q[sl, :].T),
                "wkT": _round_tf32(W_k[sl, :].T),
                "wvT": _round_tf32(W_v[sl, :].T),
                "woT": _round_tf32(W_o[:, sl].T),
                "mask": mask,
            }
        )
    res = run_bass_kernel_spmd(nc, in_maps, list(range(N_CORES)))
    outs = [res.results[i]["out"] for i in range(N_CORES)]
    full = np.stack(
        [outs[0] + outs[1] + outs[2] + outs[3], outs[4] + outs[5] + outs[6] + outs[7]]
    )
    return full.astype(np.float32)
